# revision 25
# baseline (speedup 1.0000x reference)
"""Trainium2 Bass kernel for the 9-layer dense MLP (dropout-mask training forward).

Strategy (pure data parallel, 8 cores, 8192 batch rows each):
  - Activations transposed on-chip: features on partitions, batch cols on free dim.
    Each layer computes zT = W^T @ hT via nc.tensor.matmul(out, lhsT=W, rhs=hT).
  - fp16 weights/activations/masks (fp32 PSUM accumulation), fp32 biases + output.
  - Big layers (1-5) processed in 4 blocks of 2048 batch cols; weight-major matmul
    runs (one LDWEIGHTS per (layer, outblock, kblock) after post-pass dedup).
  - PSUM as [128, 1024] two-bank tiles (4 rotating); drains are 1024-col fused
    bias+relu ops spread across ScalarE/DVE/GpSimd; dropout-mask multiplies are
    2048-col DVE ops (fp16 2x perf mode), with GpSimd taking a share.
  - Small layers (6-9) run once at the end over all 8192 cols, weight-major over
    16 subtiles (software-pipelined, partition-packed weights via tile_position),
    so there is no serial per-block ladder tail.
"""

import sys

sys.path.insert(0, "/opt/trn_rl_repo")

import numpy as np

DIMS = [256, 128, 256, 512, 256, 128, 64, 32, 16, 10]
NCORES = 8
BATCH = 65536
SHARD = BATCH // NCORES  # 8192
MSUB = 512               # matmul moving-dim tile
BLK = 2048               # block columns
NBLK = SHARD // BLK      # 4
NSUB = BLK // MSUB       # 4

# pack chunk layout (each chunk = 128 partitions x 8192 cols, fp16):
#   0,1: xT        2: m1        3,4: m2      5-8: m3
#   9,10: m4       11: m5       12: m6/m7/m8 partition-packed at rows 0/64/96
NPACK = 13

_PROG = {}


def _raise_sbuf_cap():
    # tile_utils.max_sbuf_usage is a stale 192KB constant; cayman has 208KB usable.
    import concourse.tile_utils as tu

    if getattr(tu, "max_sbuf_usage", 0) < 206 * 1024:
        tu.max_sbuf_usage = 206 * 1024


def _dedup_ldweights(nc):
    """Remove back-to-back redundant LDWEIGHTS (same stationary operand) so
    consecutive same-weight matmuls pipeline on the PE. Only drops LDW
    instructions that carry no semaphore waits/updates."""
    removed = 0
    for fn in nc.m.functions:
        for blk in fn.blocks:
            il = blk.instructions
            keep, last_sig = [], None
            for inst in il:
                nm = type(inst).__name__
                if nm == "InstLdweights":
                    sig = (str(inst.ins[0]), str(inst.is_transpose), str(inst.perf_mode),
                           str(getattr(inst, "tile_position", None)))
                    si = inst.sync_info
                    clean = si is None or (not si.on_wait and not si.on_update)
                    if sig == last_sig and clean:
                        removed += 1
                        continue
                    last_sig = sig
                keep.append(inst)
            if removed and len(keep) != len(il):
                while il:
                    il.pop()
                il.extend(keep)
    return removed


def _build_program():
    import concourse.bass as bass
    import concourse.tile as tile
    from concourse import bacc, mybir

    _raise_sbuf_cap()

    f16 = mybir.dt.float16
    f32 = mybir.dt.float32
    RELU = mybir.ActivationFunctionType.Relu
    IDENT = mybir.ActivationFunctionType.Identity
    ADD = mybir.AluOpType.add
    MAX = mybir.AluOpType.max

    nc = bacc.Bacc("TRN2", target_bir_lowering=False, debug=False, num_devices=NCORES)

    pack_d = nc.dram_tensor("pack", [128, NPACK, SHARD], f16, kind="ExternalInput").ap()
    # all weights in one host-laid-out fp16 blob, all biases in one fp32 blob
    wb_d = nc.dram_tensor("WB", [128, 2944], f16, kind="ExternalInput").ap()
    bb_d = nc.dram_tensor("BB", [128, 12], f32, kind="ExternalInput").ap()
    out_d = nc.dram_tensor("outT", [10, SHARD], f32, kind="ExternalOutput").ap()

    with tile.TileContext(nc) as tc:
        with (
            tc.tile_pool(name="wpool", bufs=1) as wp,
            tc.tile_pool(name="mk", bufs=2) as mkp,
            tc.tile_pool(name="hr", bufs=1) as hrp,
            tc.tile_pool(name="glob", bufs=1) as gp,
            tc.tile_pool(name="osb", bufs=2) as outp,
            tc.tile_pool(name="ps", bufs=4, space="PSUM") as psp,
        ):
            wall = wp.tile([128, 2944], f16, tag="wall")
            ball = wp.tile([128, 12], f32, tag="ball")
            # blob column offsets: w1@0(256) w2@256(256) w3@512(1024) w4@1536(1024)
            #   w5@2560(256) w6@2816(64) w789@2880(64: W7 r0-63 c0-31, W8 r64-95
            #   c32-47, W9 r96-111 c48-57)
            WOFF = {1: 0, 2: 256, 3: 512, 4: 1536, 5: 2560, 6: 2816, 789: 2880}
            w789 = wall[:, WOFF[789]:WOFF[789] + 64]
            b15 = ball[:, 0:10]
            b678 = ball[:, 10:11]
            b9 = ball[0:10, 11:12]

            def wslice(l, k, c, N):
                base = WOFF[l] + k * N
                return wall[:, base + c * 128: base + (c + 1) * 128]

            # persistent tiles for the tail ladder
            h5all = gp.tile([128, 1, SHARD], f16, tag="h5all")
            h678 = gp.tile([128, 1, SHARD], f16, tag="h678")
            m678 = gp.tile([128, 1, SHARD], f16, tag="m678")

            # explicit drain engine per call site (PSUM reads: ACT/DVE only)
            def drain_relu(eng, dst, zsrc, bias_ap):
                if eng == "act":
                    nc.scalar.activation(dst, zsrc, RELU, bias=bias_ap)
                else:
                    nc.vector.tensor_scalar(dst, zsrc, bias_ap, 0.0, ADD, MAX)

            def mask_mul(eng, dst, src, msrc):
                if eng == "gps":
                    nc.gpsimd.tensor_mul(dst, src, msrc)
                else:
                    nc.vector.tensor_mul(dst, src, msrc)

            # w1+w2 first so L1's first LDW fires ~0.5us in; rest follows
            nc.sync.dma_start(wall[:, 0:512], wb_d[:, 0:512])
            nc.sync.dma_start(ball[:], bb_d[:])
            nc.sync.dma_start(wall[:, 512:2944], wb_d[:, 512:2944])

            for b in range(NBLK):
                bs = bass.ts(b, BLK)
                pkx = mkp.tile([128, 2, BLK], f16, tag="pkx", name=f"pkx_{b}")
                m1 = mkp.tile([128, 1, BLK], f16, tag="m1", name=f"m1_{b}")
                m2 = mkp.tile([128, 2, BLK], f16, tag="m2", name=f"m2_{b}")
                m3 = mkp.tile([128, 4, BLK], f16, tag="m3", name=f"m3_{b}")
                m4 = mkp.tile([128, 2, BLK], f16, tag="m4", name=f"m4_{b}", bufs=1)
                m5 = mkp.tile([128, 1, BLK], f16, tag="m5", name=f"m5_{b}", bufs=1)
                nc.sync.dma_start(pkx[:, 0:1, :], pack_d[:, 0:1, bs])
                nc.sync.dma_start(pkx[:, 1:2, :], pack_d[:, 1:2, bs])
                nc.sync.dma_start(m1[:], pack_d[:, 2:3, bs])
                nc.sync.dma_start(m2[:], pack_d[:, 3:5, bs])
                nc.sync.dma_start(m3[:], pack_d[:, 5:9, bs])
                nc.sync.dma_start(m4[:], pack_d[:, 9:11, bs])
                nc.sync.dma_start(m5[:], pack_d[:, 11:12, bs])
                if b == 1:
                    # global chunk for the tail ladder; issued mid-stream so it
                    # never delays block-0 startup
                    nc.sync.dma_start(m678[:], pack_d[:, 12:13, :])

                layer_cfg = [
                    # (Kc, wl, wN, Cc, mask_tile, bias_off, tag)
                    (2, 1, 128, 1, m1, 0, "hr1"),
                    (1, 2, 256, 2, m2, 1, "hr2"),
                    (2, 3, 512, 4, m3, 3, "hr3"),
                    (4, 4, 256, 2, m4, 7, "hr4"),
                    (2, 5, 128, 1, m5, 9, "hr5"),
                ]
                prev_hm = pkx
                for li, (Kc, wl, wN, Cc, mt, boff, hrtag) in enumerate(layer_cfg):
                    last = (li == 4)
                    if last:
                        hr = None  # L5 writes straight into h5all
                    else:
                        hr = hrp.tile([128, Cc, BLK], f16, tag=hrtag,
                                      name=hrtag + f"_{b}", bufs=1)
                    for c in range(Cc):
                        # two [128,1024] two-bank psum tiles per c-group
                        zs = [psp.tile([128, 1024], f32, tag="ps",
                                       name=f"z_{hrtag}_{b}_{c}_{h}") for h in range(2)]
                        for k in range(Kc):
                            wap = wslice(wl, k, c, wN)
                            for t in range(NSUB):
                                nc.tensor.matmul(
                                    zs[t // 2][:, (t % 2) * MSUB:(t % 2 + 1) * MSUB],
                                    wap, prev_hm[:, k, bass.ts(t, MSUB)],
                                    start=(k == 0), stop=(k == Kc - 1))
                        bias_ap = b15[:, boff + c:boff + c + 1]
                        if li == 0:
                            # L1 is only 8 MMs: shave epilogue latency with
                            # 512-col drains + 1024-col masks so L2 starts fast
                            for t in range(NSUB):
                                drain_relu("act" if t % 2 == 0 else "dve",
                                           hr[:, c, bass.ts(t, MSUB)],
                                           zs[t // 2][:, (t % 2) * MSUB:
                                                      (t % 2 + 1) * MSUB],
                                           bias_ap)
                                if t % 2 == 1:
                                    hs = slice((t - 1) * MSUB, (t + 1) * MSUB)
                                    mask_mul("dve", hr[:, c, hs], hr[:, c, hs],
                                             mt[:, c, hs])
                            prev_hm = hr
                            continue
                        for h in range(2):
                            if last:
                                dst = h5all[:, 0, b * BLK + h * 1024:
                                            b * BLK + (h + 1) * 1024]
                            else:
                                dst = hr[:, c, h * 1024:(h + 1) * 1024]
                            # L5: all-ACT (DVE carries masks; GPS takes L5 mask)
                            drain_relu("act" if (last or h == 0) else "dve",
                                       dst, zs[h][:], bias_ap)
                        # mask emitted immediately after its drains so it runs
                        # early in the DVE queue (consumer MMs wait on it).
                        # L5 masks go to GpSimd: their consumer (tail ladder)
                        # is far away, so the slow engine is off-critical-path.
                        if last:
                            mask_mul("gps", h5all[:, 0, bs], h5all[:, 0, bs],
                                     mt[:, c, :])
                        else:
                            mask_mul("dve", hr[:, c, :], hr[:, c, :], mt[:, c, :])
                    prev_hm = hr if not last else None

            # ---- tail ladder: layers 6-8 over all 8192 cols, then L9 ----
            import os
            bis = int(os.environ.get("BISECT", "0"))
            # bis: 0=full, 1=no ladder/L9, 2=ladder no L9, 3=step0 only no L9
            do_ladder = bis != 1
            n_steps = {0: 3, 1: 0, 2: 3, 3: 1}[bis]
            do_l9 = bis == 0
            NT = SHARD // MSUB  # 16 subtiles
            lad_cfg = [] if not do_ladder else [
                # (p0, p1, wap, tile_pos, in_rows (None => h5all full), )
                (0, 64, wall[:, WOFF[6]:WOFF[6] + 64], None, None),
                (64, 96, w789[0:64, 0:32], (0, 64), (0, 64)),
                (96, 112, w789[64:96, 32:48], (64, 96), (64, 96)),
            ]
            for step, (p0, p1, wap, tile_pos, brange) in enumerate(lad_cfg[:n_steps]):
                for q in range(NT // 2):  # 8 two-subtile psum tiles -> 4 rotating
                    z = psp.tile([128, 1024], f32, tag="ps", name=f"zl_{step}_{q}")
                    for h in range(2):
                        t = q * 2 + h
                        ts_ = bass.ts(t, MSUB)
                        rhs = (h5all[:, 0, ts_] if step == 0 else
                               h678[brange[0]:brange[1], 0, ts_])
                        dst = z[p0:p1, h * MSUB:(h + 1) * MSUB]
                        if tile_pos is None:
                            nc.tensor.matmul(dst, wap, rhs, start=True, stop=True)
                        else:
                            nc.tensor.matmul(dst, wap, rhs, start=True, stop=True,
                                             tile_position=tile_pos)
                    # drain + mask immediately so subtile q's chain completes
                    # while the PE streams subtiles q+1.. of the same step
                    cs = slice(q * 1024, (q + 1) * 1024)
                    drain_relu("act" if q % 2 == 0 else "dve",
                               h678[p0:p1, 0, cs], z[p0:p1, :], b678[p0:p1, 0:1])
                    mask_mul("gps" if q % 3 == 2 else "dve",
                             h678[p0:p1, 0, cs], h678[p0:p1, 0, cs],
                             m678[p0:p1, 0, cs])

            # L9: 16 -> 10, bias only (fp32 out), 2048-col output groups
            for g in range(NT // 4 if do_l9 else 0):
                osb = outp.tile([10, 2048], f32, tag="osb", bufs=2, name=f"osb_{g}")
                for j in range(2):
                    q = g * 2 + j
                    z9 = psp.tile([128, 1024], f32, tag="ps", name=f"z9_{q}")
                    for h in range(2):
                        t = q * 2 + h
                        nc.tensor.matmul(z9[0:10, h * MSUB:(h + 1) * MSUB],
                                         w789[96:112, 48:58],
                                         h678[96:112, 0, bass.ts(t, MSUB)],
                                         start=True, stop=True, tile_position=(96, 0))
                    dst = osb[:, j * 1024:(j + 1) * 1024]
                    if q % 2 == 0:
                        nc.scalar.activation(dst, z9[0:10, :], IDENT, bias=b9[:, 0:1])
                    else:
                        nc.vector.tensor_scalar(dst, z9[0:10, :], b9[:, 0:1], None, ADD)
                nc.sync.dma_start(out_d[:, g * 2048:(g + 1) * 2048], osb[:])

    _dedup_ldweights(nc)
    nc.compile()
    return nc


def _get_program():
    if "nc" not in _PROG:
        _PROG["nc"] = _build_program()
    return _PROG["nc"]


def _host_prep(inputs):
    """Build per-core input maps (numpy only)."""
    x = np.asarray(inputs["x"], dtype=np.float32)
    Ws = [np.asarray(inputs[f"W{i}"], dtype=np.float32) for i in range(1, 10)]
    bs = [np.asarray(inputs[f"b{i}"], dtype=np.float32) for i in range(1, 10)]
    ms = [np.asarray(inputs[f"m{i}"], dtype=np.float32) for i in range(1, 9)]

    # fold dropout scale into next layer's weights; binarize masks
    Wf = [Ws[0]]
    for i in range(1, 9):
        s = float(ms[i - 1].max())
        if s <= 0.0:  # degenerate all-dropped mask; keep weights unscaled
            s = 1.0
        Wf.append(Ws[i] * np.float32(s))

    # weight blob: w1@0 w2@256 w3@512 w4@1536 w5@2560 w6@2816 w789@2880
    WOFF = {1: 0, 2: 256, 3: 512, 4: 1536, 5: 2560, 6: 2816, 789: 2880}
    wb = np.zeros((128, 2944), dtype=np.float16)
    for l in range(1, 7):
        W = Wf[l - 1]
        K, N = W.shape
        for k in range((K + 127) // 128):
            blk = W[k * 128:(k + 1) * 128].astype(np.float16)
            wb[: blk.shape[0], WOFF[l] + k * N: WOFF[l] + k * N + N] = blk
    wb[0:64, 2880:2912] = Wf[6].astype(np.float16)    # W7
    wb[64:96, 2912:2928] = Wf[7].astype(np.float16)   # W8
    wb[96:112, 2928:2938] = Wf[8].astype(np.float16)  # W9
    bb = np.zeros((128, 12), dtype=np.float32)
    bb[:, 0] = bs[0]
    bb[:, 1], bb[:, 2] = bs[1][0:128], bs[1][128:256]
    for c in range(4):
        bb[:, 3 + c] = bs[2][c * 128:(c + 1) * 128]
    bb[:, 7], bb[:, 8] = bs[3][0:128], bs[3][128:256]
    bb[:, 9] = bs[4]
    bb[0:64, 10], bb[64:96, 10], bb[96:112, 10] = bs[5], bs[6], bs[7]
    bb[0:10, 11] = bs[8]
    shared = {"WB": wb, "BB": bb}

    in_maps = []
    for c in range(NCORES):
        sl = slice(c * SHARD, (c + 1) * SHARD)
        pack = np.zeros((128, NPACK, SHARD), dtype=np.float16)
        xT = x[sl].T  # (256, SHARD)
        pack[:, 0, :] = xT[0:128]
        pack[:, 1, :] = xT[128:256]
        mT = [None] + [(ms[i][sl] != 0).T.astype(np.float16) for i in range(8)]  # 1-indexed
        pack[:, 2, :] = mT[1]
        pack[:, 3, :], pack[:, 4, :] = mT[2][0:128], mT[2][128:256]
        for k in range(4):
            pack[:, 5 + k, :] = mT[3][k * 128:(k + 1) * 128]
        pack[:, 9, :], pack[:, 10, :] = mT[4][0:128], mT[4][128:256]
        pack[:, 11, :] = mT[5]
        pack[0:64, 12, :] = mT[6]
        pack[64:96, 12, :] = mT[7]
        pack[96:112, 12, :] = mT[8]
        in_maps.append({"pack": pack, **shared})
    return in_maps


def kernel(**inputs) -> np.ndarray:
    from concourse.bass_utils import run_bass_kernel_spmd

    nc = _get_program()
    in_maps = _host_prep(inputs)
    res = run_bass_kernel_spmd(nc, in_maps, list(range(NCORES)))
    out = np.empty((BATCH, DIMS[-1]), dtype=np.float32)
    for c in range(NCORES):
        out[c * SHARD:(c + 1) * SHARD, :] = res.results[c]["outT"].T
    return out


# revision 26
# speedup vs baseline: 1.1147x; 1.1147x over previous
"""Trainium2 Bass kernel for the 9-layer dense MLP (dropout-mask training forward).

Strategy (pure data parallel, 8 cores, 8192 batch rows each):
  - Activations transposed on-chip: features on partitions, batch cols on free dim.
    Each layer computes zT = W^T @ hT via nc.tensor.matmul(out, lhsT=W, rhs=hT).
  - fp16 weights/activations/masks (fp32 PSUM accumulation), fp32 biases + output.
  - Big layers (1-5) processed in 4 blocks of 2048 batch cols; weight-major matmul
    runs (one LDWEIGHTS per (layer, outblock, kblock) after post-pass dedup).
  - PSUM as [128, 1024] two-bank tiles (4 rotating); drains are 1024-col fused
    bias+relu ops spread across ScalarE/DVE/GpSimd; dropout-mask multiplies are
    2048-col DVE ops (fp16 2x perf mode), with GpSimd taking a share.
  - Small layers (6-9) run once at the end over all 8192 cols, weight-major over
    16 subtiles (software-pipelined, partition-packed weights via tile_position),
    so there is no serial per-block ladder tail.
"""

import sys

sys.path.insert(0, "/opt/trn_rl_repo")

import numpy as np

DIMS = [256, 128, 256, 512, 256, 128, 64, 32, 16, 10]
NCORES = 8
BATCH = 65536
SHARD = BATCH // NCORES  # 8192
MSUB = 512               # matmul moving-dim tile
BLK = 2048               # block columns
NBLK = SHARD // BLK      # 4
NSUB = BLK // MSUB       # 4

# pack chunk layout (each chunk = 128 partitions x 8192 cols, fp16):
#   0,1: xT        2: m1        3,4: m2      5-8: m3
#   9,10: m4       11: m5       12: m6/m7/m8 partition-packed at rows 0/64/96
NPACK = 13

_PROG = {}


def _raise_sbuf_cap():
    # tile_utils.max_sbuf_usage is a stale 192KB constant; cayman has 208KB usable.
    import concourse.tile_utils as tu

    if getattr(tu, "max_sbuf_usage", 0) < 206 * 1024:
        tu.max_sbuf_usage = 206 * 1024


def _dedup_ldweights(nc):
    """Remove back-to-back redundant LDWEIGHTS (same stationary operand) so
    consecutive same-weight matmuls pipeline on the PE. Only drops LDW
    instructions that carry no semaphore waits/updates."""
    removed = 0
    for fn in nc.m.functions:
        for blk in fn.blocks:
            il = blk.instructions
            keep, last_sig = [], None
            for inst in il:
                nm = type(inst).__name__
                if nm == "InstLdweights":
                    sig = (str(inst.ins[0]), str(inst.is_transpose), str(inst.perf_mode),
                           str(getattr(inst, "tile_position", None)))
                    si = inst.sync_info
                    clean = si is None or (not si.on_wait and not si.on_update)
                    if sig == last_sig and clean:
                        removed += 1
                        continue
                    last_sig = sig
                keep.append(inst)
            if removed and len(keep) != len(il):
                while il:
                    il.pop()
                il.extend(keep)
    return removed


def _build_program():
    import concourse.bass as bass
    import concourse.tile as tile
    from concourse import bacc, mybir

    _raise_sbuf_cap()

    f16 = mybir.dt.float16
    f32 = mybir.dt.float32
    RELU = mybir.ActivationFunctionType.Relu
    IDENT = mybir.ActivationFunctionType.Identity
    ADD = mybir.AluOpType.add
    MAX = mybir.AluOpType.max

    nc = bacc.Bacc("TRN2", target_bir_lowering=False, debug=False, num_devices=NCORES)

    pack_d = nc.dram_tensor("pack", [128, NPACK, SHARD], f16, kind="ExternalInput").ap()
    # all weights in one host-laid-out fp16 blob, all biases in one fp32 blob
    wb_d = nc.dram_tensor("WB", [128, 2944], f16, kind="ExternalInput").ap()
    bb_d = nc.dram_tensor("BB", [128, 12], f32, kind="ExternalInput").ap()
    out_d = nc.dram_tensor("outT", [10, SHARD], f32, kind="ExternalOutput").ap()

    with tile.TileContext(nc) as tc:
        with (
            tc.tile_pool(name="wpool", bufs=1) as wp,
            tc.tile_pool(name="mk", bufs=2) as mkp,
            tc.tile_pool(name="hr", bufs=1) as hrp,
            tc.tile_pool(name="glob", bufs=1) as gp,
            tc.tile_pool(name="osb", bufs=2) as outp,
            tc.tile_pool(name="ps", bufs=4, space="PSUM") as psp,
        ):
            wall = wp.tile([128, 2944], f16, tag="wall")
            ball = wp.tile([128, 12], f32, tag="ball")
            # blob column offsets: w1@0(256) w2@256(256) w3@512(1024) w4@1536(1024)
            #   w5@2560(256) w6@2816(64) w789@2880(64: W7 r0-63 c0-31, W8 r64-95
            #   c32-47, W9 r96-111 c48-57)
            WOFF = {1: 0, 2: 256, 3: 512, 4: 1536, 5: 2560, 6: 2816, 789: 2880}
            w789 = wall[:, WOFF[789]:WOFF[789] + 64]
            b15 = ball[:, 0:10]
            b678 = ball[:, 10:11]
            b9 = ball[0:10, 11:12]

            def wslice(l, k, c, N):
                base = WOFF[l] + k * N
                return wall[:, base + c * 128: base + (c + 1) * 128]

            # persistent tiles for the tail ladder
            h5all = gp.tile([128, 1, SHARD], f16, tag="h5all")
            h678 = gp.tile([128, 1, SHARD], f16, tag="h678")
            m678 = gp.tile([128, 1, SHARD], f16, tag="m678")

            # engine-rotation for 1024-col drains (PSUM reads: ACT/DVE only)
            dr_i = [0]
            DRAIN_PAT = ("act", "dve", "act", "act", "dve", "act", "dve", "act")

            def drain_relu(dst, zsrc, bias_ap):
                eng = DRAIN_PAT[dr_i[0] % len(DRAIN_PAT)]
                dr_i[0] += 1
                if eng == "act":
                    nc.scalar.activation(dst, zsrc, RELU, bias=bias_ap)
                else:
                    nc.vector.tensor_scalar(dst, zsrc, bias_ap, 0.0, ADD, MAX)

            def mask_mul(eng, dst, src, msrc):
                if eng == "gps":
                    nc.gpsimd.tensor_mul(dst, src, msrc)
                else:
                    nc.vector.tensor_mul(dst, src, msrc)

            nc.sync.dma_start(wall[:], wb_d[:])
            nc.sync.dma_start(ball[:], bb_d[:])

            for b in range(NBLK):
                bs = bass.ts(b, BLK)
                pkx = mkp.tile([128, 2, BLK], f16, tag="pkx", name=f"pkx_{b}")
                m1 = mkp.tile([128, 1, BLK], f16, tag="m1", name=f"m1_{b}")
                m2 = mkp.tile([128, 2, BLK], f16, tag="m2", name=f"m2_{b}")
                m3 = mkp.tile([128, 4, BLK], f16, tag="m3", name=f"m3_{b}")
                m4 = mkp.tile([128, 2, BLK], f16, tag="m4", name=f"m4_{b}", bufs=1)
                m5 = mkp.tile([128, 1, BLK], f16, tag="m5", name=f"m5_{b}", bufs=1)
                nc.sync.dma_start(pkx[:], pack_d[:, 0:2, bs])
                nc.sync.dma_start(m1[:], pack_d[:, 2:3, bs])
                nc.sync.dma_start(m2[:], pack_d[:, 3:5, bs])
                nc.sync.dma_start(m3[:], pack_d[:, 5:9, bs])
                nc.sync.dma_start(m4[:], pack_d[:, 9:11, bs])
                nc.sync.dma_start(m5[:], pack_d[:, 11:12, bs])
                if b == 1:
                    # global chunk for the tail ladder; issued mid-stream so it
                    # never delays block-0 startup
                    nc.sync.dma_start(m678[:], pack_d[:, 12:13, :])

                layer_cfg = [
                    # (Kc, wl, wN, Cc, mask_tile, bias_off, tag)
                    (2, 1, 128, 1, m1, 0, "hr1"),
                    (1, 2, 256, 2, m2, 1, "hr2"),
                    (2, 3, 512, 4, m3, 3, "hr3"),
                    (4, 4, 256, 2, m4, 7, "hr4"),
                    (2, 5, 128, 1, m5, 9, "hr5"),
                ]
                prev_hm = pkx
                for li, (Kc, wl, wN, Cc, mt, boff, hrtag) in enumerate(layer_cfg):
                    last = (li == 4)
                    if last:
                        hr = None  # L5 writes straight into h5all
                    else:
                        hr = hrp.tile([128, Cc, BLK], f16, tag=hrtag,
                                      name=hrtag + f"_{b}", bufs=1)
                    for c in range(Cc):
                        # two [128,1024] two-bank psum tiles per c-group
                        zs = [psp.tile([128, 1024], f32, tag="ps",
                                       name=f"z_{hrtag}_{b}_{c}_{h}") for h in range(2)]
                        for k in range(Kc):
                            wap = wslice(wl, k, c, wN)
                            for t in range(NSUB):
                                nc.tensor.matmul(
                                    zs[t // 2][:, (t % 2) * MSUB:(t % 2 + 1) * MSUB],
                                    wap, prev_hm[:, k, bass.ts(t, MSUB)],
                                    start=(k == 0), stop=(k == Kc - 1))
                        bias_ap = b15[:, boff + c:boff + c + 1]
                        for h in range(2):
                            if last:
                                dst = h5all[:, 0, b * BLK + h * 1024:
                                            b * BLK + (h + 1) * 1024]
                            else:
                                dst = hr[:, c, h * 1024:(h + 1) * 1024]
                            drain_relu(dst, zs[h][:], bias_ap)
                        # mask emitted immediately after its drains so it runs
                        # early in the DVE queue (consumer MMs wait on it).
                        # L5 masks go to GpSimd: their consumer (tail ladder)
                        # is far away, so the slow engine is off-critical-path.
                        if last:
                            mask_mul("gps", h5all[:, 0, bs], h5all[:, 0, bs],
                                     mt[:, c, :])
                        else:
                            mask_mul("dve", hr[:, c, :], hr[:, c, :], mt[:, c, :])
                    prev_hm = hr if not last else None

            # ---- tail ladder: layers 6-8 over all 8192 cols, then L9 ----
            import os
            bis = int(os.environ.get("BISECT", "0"))
            # bis: 0=full, 1=no ladder/L9, 2=ladder no L9, 3=step0 only no L9
            do_ladder = bis != 1
            n_steps = {0: 3, 1: 0, 2: 3, 3: 1}[bis]
            do_l9 = bis == 0
            NT = SHARD // MSUB  # 16 subtiles
            lad_cfg = [] if not do_ladder else [
                # (p0, p1, wap, tile_pos, in_rows (None => h5all full), )
                (0, 64, wall[:, WOFF[6]:WOFF[6] + 64], None, None),
                (64, 96, w789[0:64, 0:32], (0, 64), (0, 64)),
                (96, 112, w789[64:96, 32:48], (64, 96), (64, 96)),
            ]
            for step, (p0, p1, wap, tile_pos, brange) in enumerate(lad_cfg[:n_steps]):
                for q in range(NT // 2):  # 8 two-subtile psum tiles -> 4 rotating
                    z = psp.tile([128, 1024], f32, tag="ps", name=f"zl_{step}_{q}")
                    for h in range(2):
                        t = q * 2 + h
                        ts_ = bass.ts(t, MSUB)
                        rhs = (h5all[:, 0, ts_] if step == 0 else
                               h678[brange[0]:brange[1], 0, ts_])
                        dst = z[p0:p1, h * MSUB:(h + 1) * MSUB]
                        if tile_pos is None:
                            nc.tensor.matmul(dst, wap, rhs, start=True, stop=True)
                        else:
                            nc.tensor.matmul(dst, wap, rhs, start=True, stop=True,
                                             tile_position=tile_pos)
                    # drain + mask immediately so subtile q's chain completes
                    # while the PE streams subtiles q+1.. of the same step
                    cs = slice(q * 1024, (q + 1) * 1024)
                    drain_relu(h678[p0:p1, 0, cs], z[p0:p1, :], b678[p0:p1, 0:1])
                    mask_mul("dve", h678[p0:p1, 0, cs], h678[p0:p1, 0, cs],
                             m678[p0:p1, 0, cs])

            # L9: 16 -> 10, bias only (fp32 out)
            for q in range(NT // 2 if do_l9 else 0):
                z9 = psp.tile([128, 1024], f32, tag="ps", name=f"z9_{q}")
                for h in range(2):
                    t = q * 2 + h
                    nc.tensor.matmul(z9[0:10, h * MSUB:(h + 1) * MSUB],
                                     w789[96:112, 48:58],
                                     h678[96:112, 0, bass.ts(t, MSUB)],
                                     start=True, stop=True, tile_position=(96, 0))
                osb = outp.tile([10, 1024], f32, tag="osb", bufs=2, name=f"osb_{q}")
                nc.scalar.activation(osb[:], z9[0:10, :], IDENT, bias=b9[:, 0:1])
                nc.sync.dma_start(out_d[:, q * 1024:(q + 1) * 1024], osb[:])

    _dedup_ldweights(nc)
    nc.compile()
    return nc


def _get_program():
    if "nc" not in _PROG:
        _PROG["nc"] = _build_program()
    return _PROG["nc"]


def _host_prep(inputs):
    """Build per-core input maps (numpy only)."""
    x = np.asarray(inputs["x"], dtype=np.float32)
    Ws = [np.asarray(inputs[f"W{i}"], dtype=np.float32) for i in range(1, 10)]
    bs = [np.asarray(inputs[f"b{i}"], dtype=np.float32) for i in range(1, 10)]
    ms = [np.asarray(inputs[f"m{i}"], dtype=np.float32) for i in range(1, 9)]

    # fold dropout scale into next layer's weights; binarize masks
    Wf = [Ws[0]]
    for i in range(1, 9):
        s = float(ms[i - 1].max())
        if s <= 0.0:  # degenerate all-dropped mask; keep weights unscaled
            s = 1.0
        Wf.append(Ws[i] * np.float32(s))

    # weight blob: w1@0 w2@256 w3@512 w4@1536 w5@2560 w6@2816 w789@2880
    WOFF = {1: 0, 2: 256, 3: 512, 4: 1536, 5: 2560, 6: 2816, 789: 2880}
    wb = np.zeros((128, 2944), dtype=np.float16)
    for l in range(1, 7):
        W = Wf[l - 1]
        K, N = W.shape
        for k in range((K + 127) // 128):
            blk = W[k * 128:(k + 1) * 128].astype(np.float16)
            wb[: blk.shape[0], WOFF[l] + k * N: WOFF[l] + k * N + N] = blk
    wb[0:64, 2880:2912] = Wf[6].astype(np.float16)    # W7
    wb[64:96, 2912:2928] = Wf[7].astype(np.float16)   # W8
    wb[96:112, 2928:2938] = Wf[8].astype(np.float16)  # W9
    bb = np.zeros((128, 12), dtype=np.float32)
    bb[:, 0] = bs[0]
    bb[:, 1], bb[:, 2] = bs[1][0:128], bs[1][128:256]
    for c in range(4):
        bb[:, 3 + c] = bs[2][c * 128:(c + 1) * 128]
    bb[:, 7], bb[:, 8] = bs[3][0:128], bs[3][128:256]
    bb[:, 9] = bs[4]
    bb[0:64, 10], bb[64:96, 10], bb[96:112, 10] = bs[5], bs[6], bs[7]
    bb[0:10, 11] = bs[8]
    shared = {"WB": wb, "BB": bb}

    in_maps = []
    for c in range(NCORES):
        sl = slice(c * SHARD, (c + 1) * SHARD)
        pack = np.zeros((128, NPACK, SHARD), dtype=np.float16)
        xT = x[sl].T  # (256, SHARD)
        pack[:, 0, :] = xT[0:128]
        pack[:, 1, :] = xT[128:256]
        mT = [None] + [(ms[i][sl] != 0).T.astype(np.float16) for i in range(8)]  # 1-indexed
        pack[:, 2, :] = mT[1]
        pack[:, 3, :], pack[:, 4, :] = mT[2][0:128], mT[2][128:256]
        for k in range(4):
            pack[:, 5 + k, :] = mT[3][k * 128:(k + 1) * 128]
        pack[:, 9, :], pack[:, 10, :] = mT[4][0:128], mT[4][128:256]
        pack[:, 11, :] = mT[5]
        pack[0:64, 12, :] = mT[6]
        pack[64:96, 12, :] = mT[7]
        pack[96:112, 12, :] = mT[8]
        in_maps.append({"pack": pack, **shared})
    return in_maps


def kernel(**inputs) -> np.ndarray:
    from concourse.bass_utils import run_bass_kernel_spmd

    nc = _get_program()
    in_maps = _host_prep(inputs)
    res = run_bass_kernel_spmd(nc, in_maps, list(range(NCORES)))
    out = np.empty((BATCH, DIMS[-1]), dtype=np.float32)
    for c in range(NCORES):
        out[c * SHARD:(c + 1) * SHARD, :] = res.results[c]["outT"].T
    return out


# revision 28
# speedup vs baseline: 1.1488x; 1.0306x over previous
"""Trainium2 Bass kernel for the 9-layer dense MLP (dropout-mask training forward).

Strategy (pure data parallel, 8 cores, 8192 batch rows each):
  - Activations transposed on-chip: features on partitions, batch cols on free dim.
    Each layer computes zT = W^T @ hT via nc.tensor.matmul(out, lhsT=W, rhs=hT).
  - fp16 weights/activations/masks (fp32 PSUM accumulation), fp32 biases + output.
  - Big layers (1-5) processed in 4 blocks of 2048 batch cols; weight-major matmul
    runs (one LDWEIGHTS per (layer, outblock, kblock) after post-pass dedup).
  - PSUM as [128, 1024] two-bank tiles (4 rotating); drains are 1024-col fused
    bias+relu ops spread across ScalarE/DVE/GpSimd; dropout-mask multiplies are
    2048-col DVE ops (fp16 2x perf mode), with GpSimd taking a share.
  - Small layers (6-9) run once at the end over all 8192 cols, weight-major over
    16 subtiles (software-pipelined, partition-packed weights via tile_position),
    so there is no serial per-block ladder tail.
"""

import sys

sys.path.insert(0, "/opt/trn_rl_repo")

import numpy as np

DIMS = [256, 128, 256, 512, 256, 128, 64, 32, 16, 10]
NCORES = 8
BATCH = 65536
SHARD = BATCH // NCORES  # 8192
MSUB = 512               # matmul moving-dim tile
BLK = 2048               # block columns
NBLK = SHARD // BLK      # 4
NSUB = BLK // MSUB       # 4

# pack chunk layout (each chunk = 128 partitions x 8192 cols, fp16):
#   0,1: xT        2: m1        3,4: m2      5-8: m3
#   9,10: m4       11: m5       12: m6/m7/m8 partition-packed at rows 0/64/96
NPACK = 13

_PROG = {}


def _raise_sbuf_cap():
    # tile_utils.max_sbuf_usage is a stale 192KB constant; cayman has 208KB usable.
    import concourse.tile_utils as tu

    if getattr(tu, "max_sbuf_usage", 0) < 206 * 1024:
        tu.max_sbuf_usage = 206 * 1024


def _dedup_ldweights(nc):
    """Remove back-to-back redundant LDWEIGHTS (same stationary operand) so
    consecutive same-weight matmuls pipeline on the PE. Only drops LDW
    instructions that carry no semaphore waits/updates."""
    removed = 0
    for fn in nc.m.functions:
        for blk in fn.blocks:
            il = blk.instructions
            keep, last_sig = [], None
            for inst in il:
                nm = type(inst).__name__
                if nm == "InstLdweights":
                    sig = (str(inst.ins[0]), str(inst.is_transpose), str(inst.perf_mode),
                           str(getattr(inst, "tile_position", None)))
                    si = inst.sync_info
                    clean = si is None or (not si.on_wait and not si.on_update)
                    if sig == last_sig and clean:
                        removed += 1
                        continue
                    last_sig = sig
                keep.append(inst)
            if removed and len(keep) != len(il):
                while il:
                    il.pop()
                il.extend(keep)
    return removed


def _build_program():
    import concourse.bass as bass
    import concourse.tile as tile
    from concourse import bacc, mybir

    _raise_sbuf_cap()

    f16 = mybir.dt.float16
    f32 = mybir.dt.float32
    RELU = mybir.ActivationFunctionType.Relu
    IDENT = mybir.ActivationFunctionType.Identity
    ADD = mybir.AluOpType.add
    MAX = mybir.AluOpType.max

    nc = bacc.Bacc("TRN2", target_bir_lowering=False, debug=False, num_devices=NCORES)

    pack_d = nc.dram_tensor("pack", [128, NPACK, SHARD], f16, kind="ExternalInput").ap()
    # all weights in one host-laid-out fp16 blob, all biases in one fp32 blob
    wb_d = nc.dram_tensor("WB", [128, 2944], f16, kind="ExternalInput").ap()
    bb_d = nc.dram_tensor("BB", [128, 12], f32, kind="ExternalInput").ap()
    out_d = nc.dram_tensor("outT", [10, SHARD], f32, kind="ExternalOutput").ap()

    with tile.TileContext(nc) as tc:
        with (
            tc.tile_pool(name="wpool", bufs=1) as wp,
            tc.tile_pool(name="mk", bufs=2) as mkp,
            tc.tile_pool(name="hr", bufs=1) as hrp,
            tc.tile_pool(name="glob", bufs=1) as gp,
            tc.tile_pool(name="osb", bufs=2) as outp,
            tc.tile_pool(name="ps", bufs=4, space="PSUM") as psp,
        ):
            wall = wp.tile([128, 2944], f16, tag="wall")
            ball = wp.tile([128, 12], f32, tag="ball")
            # blob column offsets: w1@0(256) w2@256(256) w3@512(1024) w4@1536(1024)
            #   w5@2560(256) w6@2816(64) w789@2880(64: W7 r0-63 c0-31, W8 r64-95
            #   c32-47, W9 r96-111 c48-57)
            WOFF = {1: 0, 2: 256, 3: 512, 4: 1536, 5: 2560, 6: 2816, 789: 2880}
            w789 = wall[:, WOFF[789]:WOFF[789] + 64]
            b15 = ball[:, 0:10]
            b678 = ball[:, 10:11]
            b9 = ball[0:10, 11:12]

            def wslice(l, k, c, N):
                base = WOFF[l] + k * N
                return wall[:, base + c * 128: base + (c + 1) * 128]

            # persistent tiles for the tail ladder
            h5all = gp.tile([128, 1, SHARD], f16, tag="h5all")
            h678 = gp.tile([128, 1, SHARD], f16, tag="h678")
            m678 = gp.tile([128, 1, SHARD], f16, tag="m678")

            # engine-rotation for 1024-col drains (PSUM reads: ACT/DVE only)
            dr_i = [0]
            DRAIN_PAT = ("act", "dve", "act", "act", "dve", "act", "dve", "act")

            def drain_relu(dst, zsrc, bias_ap):
                eng = DRAIN_PAT[dr_i[0] % len(DRAIN_PAT)]
                dr_i[0] += 1
                if eng == "act":
                    nc.scalar.activation(dst, zsrc, RELU, bias=bias_ap)
                else:
                    nc.vector.tensor_scalar(dst, zsrc, bias_ap, 0.0, ADD, MAX)

            def drain_relu2(eng, dst, zsrc, bias_ap):
                if eng == "act":
                    nc.scalar.activation(dst, zsrc, RELU, bias=bias_ap)
                else:
                    nc.vector.tensor_scalar(dst, zsrc, bias_ap, 0.0, ADD, MAX)

            def mask_mul(eng, dst, src, msrc):
                if eng == "gps":
                    nc.gpsimd.tensor_mul(dst, src, msrc)
                else:
                    nc.vector.tensor_mul(dst, src, msrc)

            nc.sync.dma_start(wall[:], wb_d[:])
            nc.sync.dma_start(ball[:], bb_d[:])

            for b in range(NBLK):
                bs = bass.ts(b, BLK)
                pkx = mkp.tile([128, 2, BLK], f16, tag="pkx", name=f"pkx_{b}")
                m1 = mkp.tile([128, 1, BLK], f16, tag="m1", name=f"m1_{b}")
                m2 = mkp.tile([128, 2, BLK], f16, tag="m2", name=f"m2_{b}")
                m3 = mkp.tile([128, 4, BLK], f16, tag="m3", name=f"m3_{b}")
                m4 = mkp.tile([128, 2, BLK], f16, tag="m4", name=f"m4_{b}", bufs=1)
                m5 = mkp.tile([128, 1, BLK], f16, tag="m5", name=f"m5_{b}", bufs=1)
                nc.sync.dma_start(pkx[:], pack_d[:, 0:2, bs])
                nc.sync.dma_start(m1[:], pack_d[:, 2:3, bs])
                nc.sync.dma_start(m2[:], pack_d[:, 3:5, bs])
                nc.sync.dma_start(m3[:], pack_d[:, 5:9, bs])
                nc.sync.dma_start(m4[:], pack_d[:, 9:11, bs])
                nc.sync.dma_start(m5[:], pack_d[:, 11:12, bs])
                if b == 1:
                    # global chunk for the tail ladder; issued mid-stream so it
                    # never delays block-0 startup
                    nc.sync.dma_start(m678[:], pack_d[:, 12:13, :])

                layer_cfg = [
                    # (Kc, wl, wN, Cc, mask_tile, bias_off, tag)
                    (2, 1, 128, 1, m1, 0, "hr1"),
                    (1, 2, 256, 2, m2, 1, "hr2"),
                    (2, 3, 512, 4, m3, 3, "hr3"),
                    (4, 4, 256, 2, m4, 7, "hr4"),
                    (2, 5, 128, 1, m5, 9, "hr5"),
                ]
                prev_hm = pkx
                for li, (Kc, wl, wN, Cc, mt, boff, hrtag) in enumerate(layer_cfg):
                    last = (li == 4)
                    if last:
                        hr = None  # L5 writes straight into h5all
                    else:
                        hr = hrp.tile([128, Cc, BLK], f16, tag=hrtag,
                                      name=hrtag + f"_{b}", bufs=1)
                    for c in range(Cc):
                        # two [128,1024] two-bank psum tiles per c-group
                        zs = [psp.tile([128, 1024], f32, tag="ps",
                                       name=f"z_{hrtag}_{b}_{c}_{h}") for h in range(2)]
                        for k in range(Kc):
                            wap = wslice(wl, k, c, wN)
                            for t in range(NSUB):
                                nc.tensor.matmul(
                                    zs[t // 2][:, (t % 2) * MSUB:(t % 2 + 1) * MSUB],
                                    wap, prev_hm[:, k, bass.ts(t, MSUB)],
                                    start=(k == 0), stop=(k == Kc - 1))
                        bias_ap = b15[:, boff + c:boff + c + 1]
                        if li == 0:
                            # L1 is only 8 MMs: 512-col drains + 1024-col masks
                            # cut the L1->L2 dependency bubble. Explicit engines;
                            # dr_i advanced by 2 to keep v3's rotation phase for
                            # the other layers.
                            for t in range(NSUB):
                                drain_relu2("act" if t % 2 == 0 else "dve",
                                            hr[:, c, bass.ts(t, MSUB)],
                                            zs[t // 2][:, (t % 2) * MSUB:
                                                       (t % 2 + 1) * MSUB],
                                            bias_ap)
                                if t % 2 == 1:
                                    hs = slice((t - 1) * MSUB, (t + 1) * MSUB)
                                    mask_mul("dve", hr[:, c, hs], hr[:, c, hs],
                                             mt[:, c, hs])
                            dr_i[0] += 2
                            prev_hm = hr
                            continue
                        for h in range(2):
                            if last:
                                dst = h5all[:, 0, b * BLK + h * 1024:
                                            b * BLK + (h + 1) * 1024]
                            else:
                                dst = hr[:, c, h * 1024:(h + 1) * 1024]
                            drain_relu(dst, zs[h][:], bias_ap)
                        # mask emitted immediately after its drains so it runs
                        # early in the DVE queue (consumer MMs wait on it).
                        # L5 masks go to GpSimd: their consumer (tail ladder)
                        # is far away, so the slow engine is off-critical-path.
                        if last:
                            mask_mul("gps", h5all[:, 0, bs], h5all[:, 0, bs],
                                     mt[:, c, :])
                        else:
                            mask_mul("dve", hr[:, c, :], hr[:, c, :], mt[:, c, :])
                    prev_hm = hr if not last else None

            # ---- tail ladder: layers 6-8 over all 8192 cols, then L9 ----
            import os
            bis = int(os.environ.get("BISECT", "0"))
            # bis: 0=full, 1=no ladder/L9, 2=ladder no L9, 3=step0 only no L9
            do_ladder = bis != 1
            n_steps = {0: 3, 1: 0, 2: 3, 3: 1}[bis]
            do_l9 = bis == 0
            NT = SHARD // MSUB  # 16 subtiles
            lad_cfg = [] if not do_ladder else [
                # (p0, p1, wap, tile_pos, in_rows (None => h5all full), )
                (0, 64, wall[:, WOFF[6]:WOFF[6] + 64], None, None),
                (64, 96, w789[0:64, 0:32], (0, 64), (0, 64)),
                (96, 112, w789[64:96, 32:48], (64, 96), (64, 96)),
            ]
            for step, (p0, p1, wap, tile_pos, brange) in enumerate(lad_cfg[:n_steps]):
                for q in range(NT // 2):  # 8 two-subtile psum tiles -> 4 rotating
                    z = psp.tile([128, 1024], f32, tag="ps", name=f"zl_{step}_{q}")
                    for h in range(2):
                        t = q * 2 + h
                        ts_ = bass.ts(t, MSUB)
                        rhs = (h5all[:, 0, ts_] if step == 0 else
                               h678[brange[0]:brange[1], 0, ts_])
                        dst = z[p0:p1, h * MSUB:(h + 1) * MSUB]
                        if tile_pos is None:
                            nc.tensor.matmul(dst, wap, rhs, start=True, stop=True)
                        else:
                            nc.tensor.matmul(dst, wap, rhs, start=True, stop=True,
                                             tile_position=tile_pos)
                    # drain + mask immediately so subtile q's chain completes
                    # while the PE streams subtiles q+1.. of the same step
                    cs = slice(q * 1024, (q + 1) * 1024)
                    drain_relu(h678[p0:p1, 0, cs], z[p0:p1, :], b678[p0:p1, 0:1])
                    mask_mul("dve", h678[p0:p1, 0, cs], h678[p0:p1, 0, cs],
                             m678[p0:p1, 0, cs])

            # L9: 16 -> 10, bias only (fp32 out)
            for q in range(NT // 2 if do_l9 else 0):
                z9 = psp.tile([128, 1024], f32, tag="ps", name=f"z9_{q}")
                for h in range(2):
                    t = q * 2 + h
                    nc.tensor.matmul(z9[0:10, h * MSUB:(h + 1) * MSUB],
                                     w789[96:112, 48:58],
                                     h678[96:112, 0, bass.ts(t, MSUB)],
                                     start=True, stop=True, tile_position=(96, 0))
                osb = outp.tile([10, 1024], f32, tag="osb", bufs=2, name=f"osb_{q}")
                nc.scalar.activation(osb[:], z9[0:10, :], IDENT, bias=b9[:, 0:1])
                nc.sync.dma_start(out_d[:, q * 1024:(q + 1) * 1024], osb[:])

    _dedup_ldweights(nc)
    nc.compile()
    return nc


def _get_program():
    if "nc" not in _PROG:
        _PROG["nc"] = _build_program()
    return _PROG["nc"]


def _host_prep(inputs):
    """Build per-core input maps (numpy only)."""
    x = np.asarray(inputs["x"], dtype=np.float32)
    Ws = [np.asarray(inputs[f"W{i}"], dtype=np.float32) for i in range(1, 10)]
    bs = [np.asarray(inputs[f"b{i}"], dtype=np.float32) for i in range(1, 10)]
    ms = [np.asarray(inputs[f"m{i}"], dtype=np.float32) for i in range(1, 9)]

    # fold dropout scale into next layer's weights; binarize masks
    Wf = [Ws[0]]
    for i in range(1, 9):
        s = float(ms[i - 1].max())
        if s <= 0.0:  # degenerate all-dropped mask; keep weights unscaled
            s = 1.0
        Wf.append(Ws[i] * np.float32(s))

    # weight blob: w1@0 w2@256 w3@512 w4@1536 w5@2560 w6@2816 w789@2880
    WOFF = {1: 0, 2: 256, 3: 512, 4: 1536, 5: 2560, 6: 2816, 789: 2880}
    wb = np.zeros((128, 2944), dtype=np.float16)
    for l in range(1, 7):
        W = Wf[l - 1]
        K, N = W.shape
        for k in range((K + 127) // 128):
            blk = W[k * 128:(k + 1) * 128].astype(np.float16)
            wb[: blk.shape[0], WOFF[l] + k * N: WOFF[l] + k * N + N] = blk
    wb[0:64, 2880:2912] = Wf[6].astype(np.float16)    # W7
    wb[64:96, 2912:2928] = Wf[7].astype(np.float16)   # W8
    wb[96:112, 2928:2938] = Wf[8].astype(np.float16)  # W9
    bb = np.zeros((128, 12), dtype=np.float32)
    bb[:, 0] = bs[0]
    bb[:, 1], bb[:, 2] = bs[1][0:128], bs[1][128:256]
    for c in range(4):
        bb[:, 3 + c] = bs[2][c * 128:(c + 1) * 128]
    bb[:, 7], bb[:, 8] = bs[3][0:128], bs[3][128:256]
    bb[:, 9] = bs[4]
    bb[0:64, 10], bb[64:96, 10], bb[96:112, 10] = bs[5], bs[6], bs[7]
    bb[0:10, 11] = bs[8]
    shared = {"WB": wb, "BB": bb}

    in_maps = []
    for c in range(NCORES):
        sl = slice(c * SHARD, (c + 1) * SHARD)
        pack = np.zeros((128, NPACK, SHARD), dtype=np.float16)
        xT = x[sl].T  # (256, SHARD)
        pack[:, 0, :] = xT[0:128]
        pack[:, 1, :] = xT[128:256]
        mT = [None] + [(ms[i][sl] != 0).T.astype(np.float16) for i in range(8)]  # 1-indexed
        pack[:, 2, :] = mT[1]
        pack[:, 3, :], pack[:, 4, :] = mT[2][0:128], mT[2][128:256]
        for k in range(4):
            pack[:, 5 + k, :] = mT[3][k * 128:(k + 1) * 128]
        pack[:, 9, :], pack[:, 10, :] = mT[4][0:128], mT[4][128:256]
        pack[:, 11, :] = mT[5]
        pack[0:64, 12, :] = mT[6]
        pack[64:96, 12, :] = mT[7]
        pack[96:112, 12, :] = mT[8]
        in_maps.append({"pack": pack, **shared})
    return in_maps


def kernel(**inputs) -> np.ndarray:
    from concourse.bass_utils import run_bass_kernel_spmd

    nc = _get_program()
    in_maps = _host_prep(inputs)
    res = run_bass_kernel_spmd(nc, in_maps, list(range(NCORES)))
    out = np.empty((BATCH, DIMS[-1]), dtype=np.float32)
    for c in range(NCORES):
        out[c * SHARD:(c + 1) * SHARD, :] = res.results[c]["outT"].T
    return out
